# revision 24
# baseline (speedup 1.0000x reference)
"""Trainium2 Bass kernel for a 2-layer GraphConv GNN + mean-pool + linear.

Reference computation (all fp32):
    h1 = leaky_relu(segsum(w*x[src] -> dst) @ W1_rel + x @ W1_root + b1)
    h2 = leaky_relu(segsum(w*h1[src] -> dst) @ W2_rel + h1 @ W2_root + b2)
    pooled = segment_mean(h2, batch, 512)
    out = pooled @ Wl_root + bl            # [512, 8]

Distribution (8 NeuronCores):
    - Nodes in contiguous shards of 12500/core; edges assigned to the core
      owning their dst node; bf16 on-chip, fp32 PSUM accumulation.
    - W_rel is folded into the gather tables: the host premultiplies
      xt = x @ W1_rel into the layer-1 table; layer 1 computes
      ht = h1 @ W2_rel on-chip and stores THAT as the layer-2 table. The
      aggregation matmuls then accumulate the "rel" term directly into the
      z PSUM tile, and one extra matmul adds the root term W_root^T @ x_fm.
    - Gathers use batched InstDMAGatherAnt (int16 indices): edges are sorted
      by (dst block, src row); chunks of 128 edges are grouped into batches
      of <=8 chunks (1024-idx ucode cap) sharing one 32768-row table window,
      amortizing the ~1us SWDGE fixed overhead ~8x vs indirect_dma_start.
      Batches alternate between 2 SWDGE queues: a single in-order queue
      serializes the random-row transfers (~10us/batch measured); two queues
      pipeline them back to the desc-gen rate (~1.4us/batch measured).
    - Scatter-add to dst is a one-hot matmul per chunk (DVE builds
      onehot[e,s] = (iota==dib)*w, TensorE contracts into z PSUM); the bias
      rides in the root matmul via a ones-row in the feature-major activation
      tiles, and leaky-relu is one Act copy + one DVE scalar_tensor_tensor.
    - The h1 AllGather is split in two shard-half pieces (piece-permuted
      row layout): piece 0 overlaps layer-1's second half, and piece 1
      overlaps layer-2's early gather batches, whose 32768-row windows lie
      entirely inside piece 0 and so depend only on the first collective.
    - Per-graph pooling is a one-hot matmul accumulated across blocks; the
      trivial overlap-add + mean + final 64x8 linear run on host.
"""

import math

import numpy as np
import ml_dtypes

import concourse.bacc as bacc
import concourse.mybir as mybir
import concourse.tile as tile
from concourse.bass_utils import run_bass_kernel_spmd

F32 = mybir.dt.float32
B16 = mybir.dt.bfloat16
I16 = mybir.dt.int16
ALU = mybir.AluOpType
ACTF = mybir.ActivationFunctionType
NPBF = ml_dtypes.bfloat16

WL = 32768          # gather window rows (int16 index reach)
GROUP = 8           # blocks per gather group
MAXCH = 8           # max chunks per gather batch (1024-idx ucode cap)


class Cfg:
    def __init__(self, n_nodes, n_edges, d, n_graphs, n_cores=8, block=128):
        self.N = n_nodes
        self.E = n_edges
        self.D = d
        self.G = n_graphs
        self.CORES = n_cores
        self.BLOCK = block
        self.NPC = n_nodes // n_cores                    # real nodes per core
        self.NB = math.ceil(self.NPC / block)            # blocks per core
        self.NPAD = self.NB * block                      # padded nodes/core
        self.PIECE = self.NPAD // 2                      # AllGather piece rows (Q=2)
        self.TROWS = self.NPAD * n_cores                 # table rows
        self.LEAKY = 0.01


REAL_CFG = Cfg(100000, 1250000, 64, 512)


# ---------------------------------------------------------------------------
# Host-side preprocessing
# ---------------------------------------------------------------------------

def _perm_row(cfg, src):
    """Real node id -> piece-permuted table row (vectorized)."""
    c = src // cfg.NPC
    r = src - c * cfg.NPC
    q = r // cfg.PIECE
    rr = r - q * cfg.PIECE
    return q * (cfg.CORES * cfg.PIECE) + c * cfg.PIECE + rr


def preprocess(cfg, x, edge_index, weights, batch, W1_rel):
    N, E, D, CORES = cfg.N, cfg.E, cfg.D, cfg.CORES
    NPC, NB, NPAD, BLOCK = cfg.NPC, cfg.NB, cfg.NPAD, cfg.BLOCK

    src = np.asarray(edge_index[0], dtype=np.int64)
    dst = np.asarray(edge_index[1], dtype=np.int64)
    w = np.asarray(weights, dtype=np.float32)
    batch = np.asarray(batch, dtype=np.int64)

    # --- per-core edge lists sorted by (dst block, permuted src row) -------
    rowp_all = _perm_row(cfg, src)
    c_dst = dst // NPC
    per_core = []
    cnt = np.zeros((CORES, NB), dtype=np.int64)
    for c in range(CORES):
        m = c_dst == c
        s_r, d_r, w_r = rowp_all[m], dst[m] - c * NPC, w[m]
        blk = d_r // BLOCK
        order = np.lexsort((s_r, blk))
        s_r, d_r, w_r, blk = s_r[order], d_r[order], w_r[order], blk[order]
        dib = d_r - blk * BLOCK
        cnt[c] = np.bincount(blk, minlength=NB)
        per_core.append((s_r, dib, w_r, blk))

    # --- shared chunk/batch structure (SPMD: identical program) ------------
    K_b = np.maximum(1, -(-cnt.max(axis=0) // BLOCK))    # chunks per block
    Kmax = int(K_b.max())

    # per-(block, chunk) src-row extents across all cores
    BIG = np.iinfo(np.int64).max
    rmin = np.full((NB, Kmax), BIG, dtype=np.int64)
    rmax = np.full((NB, Kmax), -1, dtype=np.int64)
    for c in range(CORES):
        s_r, dib, w_r, blk = per_core[c]
        first = np.concatenate([[0], np.cumsum(cnt[c])])[:-1]
        rank = np.arange(len(s_r)) - first[blk]
        ck = rank // BLOCK
        np.minimum.at(rmin, (blk, ck), s_r)
        np.maximum.at(rmax, (blk, ck), s_r)

    SPAN_LIM = WL - 512     # safety margin under the int16 reach
    n_groups = math.ceil(NB / GROUP)
    group_blocks = [list(range(g * GROUP, min((g + 1) * GROUP, NB)))
                    for g in range(n_groups)]
    # serpentine k order: odd groups descend k, so the last band of group g
    # borders the same src band as the first batch of group g+1 and batches
    # can merge across group boundaries without blowing the window span
    cand = []
    for g in range(n_groups):
        blocks = group_blocks[g]
        Kg = int(max(K_b[b] for b in blocks))
        ks = range(Kg) if g % 2 == 0 else range(Kg - 1, -1, -1)
        cand.extend((g, b, k) for k in ks for b in blocks if k < K_b[b])

    chunk_col = np.full((NB, Kmax + 1), -1, dtype=np.int64)
    chunk_batch = np.full_like(chunk_col, -1)
    chunk_jpos = np.full_like(chunk_col, -1)
    n_cols = 0
    batches = []
    windows = []
    icols = 0
    state = {"cur": [], "mn": BIG, "mx": -1}

    def flush():
        nonlocal n_cols, icols
        cur = state["cur"]
        if not cur:
            return
        bidx = len(batches)
        for j, (g, b, k) in enumerate(cur):
            chunk_col[b, k] = n_cols + j
            chunk_batch[b, k] = bidx
            chunk_jpos[b, k] = j
        O = 0 if state["mx"] < 0 else min(state["mn"], cfg.TROWS - WL)
        assert state["mx"] - O <= WL - 1
        bt = {"id": bidx, "members": [(b, k) for (g, b, k) in cur],
              "col0": n_cols, "icol0": icols, "nch": len(cur),
              "group": cur[0][0]}
        batches.append(bt)
        windows.append(int(O))
        n_cols += len(cur)
        icols += len(cur) * (BLOCK // 16)
        state["cur"], state["mn"], state["mx"] = [], BIG, -1

    for (g, b, k) in cand:
        mn = rmin[b, k] if rmin[b, k] != BIG else None
        mx = rmax[b, k]
        nmn = state["mn"] if mn is None else min(state["mn"], mn)
        nmx = max(state["mx"], mx)
        if state["cur"] and (len(state["cur"]) >= MAXCH
                            or (nmx >= 0 and nmn != BIG
                                and nmx - nmn > SPAN_LIM)):
            flush()
            nmn = mn if mn is not None else BIG
            nmx = mx
        state["cur"].append((g, b, k))
        state["mn"], state["mx"] = nmn, nmx
    flush()
    groups = [{"blocks": group_blocks[g],
               "batches": [bt for bt in batches if bt["group"] == g]}
              for g in range(n_groups)]
    windows = np.asarray(windows, dtype=np.int64)

    # --- per-core idx/dib/wgt arrays ----------------------------------------
    in_maps = []
    icol_tot = icols
    g_base = batch[np.arange(CORES) * NPC]
    xt = (x.astype(np.float32) @ np.asarray(W1_rel, dtype=np.float32))
    x2 = np.zeros((cfg.TROWS, 2 * D), dtype=NPBF)
    rows_real = _perm_row(cfg, np.arange(N))
    x2[rows_real, :D] = xt.astype(NPBF)

    for c in range(CORES):
        s_r, dib, w_r, blk = per_core[c]
        first = np.concatenate([[0], np.cumsum(cnt[c])])[:-1]
        rank = np.arange(len(s_r)) - first[blk]
        ck = rank // BLOCK
        p = rank % BLOCK
        cols = chunk_col[blk, ck]
        bidx = chunk_batch[blk, ck]
        jpos = chunk_jpos[blk, ck]

        dib_a = np.full((BLOCK, n_cols), 200.0, dtype=np.float32)
        wgt_a = np.zeros((BLOCK, n_cols), dtype=np.float32)
        dib_a[p, cols] = dib.astype(np.float32)
        wgt_a[p, cols] = w_r

        idx16 = np.zeros((16, icol_tot), dtype=np.int16)
        i_lin = jpos * BLOCK + p
        vals = s_r - windows[bidx]
        assert vals.min() >= 0 and vals.max() <= WL - 1
        icolx = np.array([batches[int(bx)]["icol0"] for bx in bidx]) \
            + i_lin // 16
        idx16[i_lin % 16, icolx] = vals.astype(np.int16)
        src16 = np.tile(idx16, (8, 1))

        xs = x[c * NPC:(c + 1) * NPC]
        xT = np.zeros((D + 1, NPAD), dtype=NPBF)
        xT[:D, :NPC] = xs.T.astype(NPBF)
        xT[D, :] = NPBF(1.0)
        gs = np.full(NPAD, -1.0, dtype=np.float32)
        gs[:NPC] = (batch[c * NPC:(c + 1) * NPC] - g_base[c]).astype(np.float32)
        assert gs.max() < 128.0, "graph span per core exceeds 128"
        in_maps.append({
            "x2": x2,
            "xT": xT,
            "src16": src16,
            "dib": dib_a,
            "wgt": wgt_a,
            "gslot": np.ascontiguousarray(
                gs.reshape(NB, BLOCK).T),
        })

    prog = {"groups": groups, "batches": batches, "windows": windows.tolist(),
            "chunk_col": chunk_col, "K_b": K_b, "n_cols": n_cols,
            "icol_tot": icol_tot}
    return in_maps, prog, g_base


# ---------------------------------------------------------------------------
# Bass program
# ---------------------------------------------------------------------------

def build_nc(cfg, prog):
    D, CORES, NB, NPAD = cfg.D, cfg.CORES, cfg.NB, cfg.NPAD
    n_cols, icol_tot = prog["n_cols"], prog["icol_tot"]
    groups, batches, windows = prog["groups"], prog["batches"], prog["windows"]
    chunk_col = prog["chunk_col"]

    nc = bacc.Bacc("TRN2", target_bir_lowering=False, debug=False,
                   num_devices=CORES, dynamic_dma_scratch_size=32768,
                   num_swdge_queues=2)

    x2_d = nc.dram_tensor("x2", [cfg.TROWS, 2 * D], B16, kind="ExternalInput")
    xT_d = nc.dram_tensor("xT", [D + 1, NPAD], B16, kind="ExternalInput")
    src16_d = nc.dram_tensor("src16", [128, icol_tot], I16,
                             kind="ExternalInput")
    dib_d = nc.dram_tensor("dib", [128, n_cols], F32, kind="ExternalInput")
    wgt_d = nc.dram_tensor("wgt", [128, n_cols], F32, kind="ExternalInput")
    gslot_d = nc.dram_tensor("gslot", [128, NB], F32, kind="ExternalInput")
    w1o_d = nc.dram_tensor("w1o", [D + 1, D], B16, kind="ExternalInput")
    w2o_d = nc.dram_tensor("w2o", [D + 1, D], B16, kind="ExternalInput")
    w2r_d = nc.dram_tensor("w2r", [D, D], B16, kind="ExternalInput")
    iota_d = nc.dram_tensor("iota", [128, 128], B16, kind="ExternalInput")
    id64_d = nc.dram_tensor("id64", [D, D], B16, kind="ExternalInput")

    pool_d = nc.dram_tensor("pool", [128, D], F32, kind="ExternalOutput")

    h1_local = nc.dram_tensor("h1_local", [NPAD, 2 * D], B16)
    h1_full = nc.dram_tensor("h1_full", [cfg.TROWS, 2 * D], B16,
                             addr_space="Shared")

    with tile.TileContext(nc) as tc:
        with (
            tc.tile_pool(name="persist", bufs=1) as pp,
            tc.tile_pool(name="work", bufs=6) as wp,
            tc.tile_pool(name="gat", bufs=30) as gp,
            tc.tile_pool(name="psz", bufs=3, space="PSUM") as psz,
            tc.tile_pool(name="psht", bufs=2, space="PSUM") as psht,
            tc.tile_pool(name="pstp", bufs=2, space="PSUM") as pstp,
            tc.tile_pool(name="pool1", bufs=1, space="PSUM") as pool1,
        ):
            xT_s = pp.tile([D + 1, NPAD], B16, tag="xT")
            h1T_s = pp.tile([D + 1, NPAD], B16, tag="h1T")
            src16_s = pp.tile([128, icol_tot], I16, tag="src16")
            dib_s = pp.tile([128, n_cols], F32, tag="dib")
            wgt_s = pp.tile([128, n_cols], F32, tag="wgt")
            gslot_s = pp.tile([128, NB], F32, tag="gslot")
            w1o_s = pp.tile([D + 1, D], B16, tag="w1o")
            w2o_s = pp.tile([D + 1, D], B16, tag="w2o")
            w2r_s = pp.tile([D, D], B16, tag="w2r")
            iota_s = pp.tile([128, 128], B16, tag="iota")
            id64_s = pp.tile([D, D], B16, tag="id64")

            for t, d_ in [(xT_s, xT_d), (gslot_s, gslot_d), (w1o_s, w1o_d),
                          (w2o_s, w2o_d), (w2r_s, w2r_d),
                          (iota_s, iota_d), (id64_s, id64_d)]:
                nc.sync.dma_start(out=t[:], in_=d_[:, :])
            # piecewise loads so the first gathers/onehots start immediately
            for t, d_, nco in [(src16_s, src16_d, icol_tot),
                               (dib_s, dib_d, n_cols),
                               (wgt_s, wgt_d, n_cols)]:
                step = max(256, -(-nco // 8))
                c0 = 0
                first = True
                while c0 < nco:
                    c1 = min(c0 + (128 if first else step), nco)
                    nc.sync.dma_start(out=t[:, c0:c1], in_=d_[:, c0:c1])
                    c0 = c1
                    first = False
            # ones row for the layer-2 root bias trick
            nc.sync.dma_start(out=h1T_s[D:D + 1, :], in_=xT_d[D:D + 1, :])

            pool_ps = pool1.tile([128, D], F32, tag="pool")

            def gather_batches(group, table_d, gtag, col_map):
                for bt in group["batches"]:
                    gb = gp.tile([128, MAXCH, 2 * D], B16, tag=gtag)
                    nch = bt["nch"]
                    O = int(windows[bt["id"]])
                    nc.gpsimd.dma_gather(
                        gb[:, 0:nch, :],
                        table_d[O:O + WL, :],
                        src16_s[:, bt["icol0"]:bt["icol0"] + nch * 8],
                        nch * 128,
                        nch * 128,
                        2 * D,
                        queue_num=bt["id"] % 2,
                    )
                    for j in range(nch):
                        col_map[bt["col0"] + j] = (gb, j)

            def block_agg(z_ps, b, Kb, col_map):
                """accumulate aggregation matmuls for block b into z_ps."""
                for k in range(Kb):
                    col = int(chunk_col[b, k])
                    gb_t, j = col_map[col]
                    oh = wp.tile([128, 128], B16, tag="oh")
                    nc.vector.tensor_scalar(
                        out=oh[:], in0=iota_s[:],
                        scalar1=dib_s[:, col:col + 1],
                        scalar2=wgt_s[:, col:col + 1],
                        op0=ALU.is_equal, op1=ALU.mult)
                    nc.tensor.matmul(
                        out=z_ps[:],
                        lhsT=gb_t[:, j, 0:D],
                        rhs=oh[:],
                        start=(k == 0), stop=False)

            def leaky_to(dst_ap, z_ps):
                # leaky_relu(z) = max(0.01*z, z), bias already folded into z
                zb = wp.tile([D, 128], B16, tag="zb")
                nc.scalar.activation(out=zb[:], in_=z_ps[:], func=ACTF.Copy)
                nc.vector.scalar_tensor_tensor(
                    out=dst_ap, in0=zb[:], scalar=cfg.LEAKY, in1=zb[:],
                    op0=ALU.mult, op1=ALU.max)

            # ---------------- layer 1 ----------------
            tiles = {}
            for g_i, group in enumerate(groups):
                gather_batches(group, x2_d, "gb", tiles)
                for b in group["blocks"]:
                    z_ps = psz.tile([D, 128], F32, tag="z")
                    block_agg(z_ps, b, int(prog["K_b"][b]), tiles)
                    nc.tensor.matmul(out=z_ps[:], lhsT=w1o_s[:],
                                     rhs=xT_s[:, b * 128:(b + 1) * 128],
                                     start=False, stop=True)
                    hsl = h1T_s[0:D, b * 128:(b + 1) * 128]
                    leaky_to(hsl, z_ps)
                    ht_ps = psht.tile([D, 128], F32, tag="ht")
                    nc.tensor.matmul(out=ht_ps[:], lhsT=w2r_s[:], rhs=hsl,
                                     start=True, stop=True)
                    hts = wp.tile([D, 128], B16, tag="hts")
                    nc.scalar.activation(out=hts[:], in_=ht_ps[:],
                                         func=ACTF.Copy)
                    t_ps = pstp.tile([128, D], B16, tag="tp")
                    nc.tensor.transpose(out=t_ps[:], in_=hts[:],
                                        identity=id64_s[:])
                    hnm = wp.tile([128, D], B16, tag="hnm")
                    nc.scalar.activation(out=hnm[:], in_=t_ps[:],
                                         func=ACTF.Copy)
                    nc.sync.dma_start(
                        out=h1_local[b * 128:(b + 1) * 128, 0:D], in_=hnm[:])
                    if (b + 1) * 128 in (cfg.PIECE, NPAD):
                        q = 0 if (b + 1) * 128 == cfg.PIECE else 1
                        nc.gpsimd.collective_compute(
                            "AllGather",
                            ALU.bypass,
                            replica_groups=[list(range(CORES))],
                            ins=[h1_local[q * cfg.PIECE:(q + 1) * cfg.PIECE, :]],
                            outs=[h1_full[q * cfg.PIECE * CORES:
                                          (q + 1) * cfg.PIECE * CORES, :]],
                        )

            # ---------------- layer 2 ----------------
            tiles = {}
            for g_i, group in enumerate(groups):
                gather_batches(group, h1_full, "gb", tiles)
                for b in group["blocks"]:
                    z_ps = psz.tile([D, 128], F32, tag="z")
                    block_agg(z_ps, b, int(prog["K_b"][b]), tiles)
                    nc.tensor.matmul(out=z_ps[:], lhsT=w2o_s[:],
                                     rhs=h1T_s[:, b * 128:(b + 1) * 128],
                                     start=False, stop=True)
                    h2f = wp.tile([D, 128], B16, tag="h2f")
                    leaky_to(h2f[:], z_ps)
                    t_ps = pstp.tile([128, D], B16, tag="tp")
                    nc.tensor.transpose(out=t_ps[:], in_=h2f[:],
                                        identity=id64_s[:])
                    h2nm = wp.tile([128, D], B16, tag="h2nm")
                    nc.scalar.activation(out=h2nm[:], in_=t_ps[:],
                                         func=ACTF.Copy)
                    ph = wp.tile([128, 128], B16, tag="ph")
                    nc.vector.tensor_scalar(
                        out=ph[:], in0=iota_s[:],
                        scalar1=gslot_s[:, b:b + 1], scalar2=None,
                        op0=ALU.is_equal)
                    nc.tensor.matmul(out=pool_ps[:], lhsT=ph[:], rhs=h2nm[:],
                                     start=(b == 0), stop=(b == NB - 1))

            pool_s = wp.tile([128, D], F32, tag="pools")
            nc.scalar.activation(out=pool_s[:], in_=pool_ps[:], func=ACTF.Copy)
            nc.sync.dma_start(out=pool_d[:, :], in_=pool_s[:])

    nc.compile()
    return nc


# ---------------------------------------------------------------------------
# Entry point
# ---------------------------------------------------------------------------

_CACHE = {}


def _common_inputs(cfg, W1_root, W2_root, W2_rel, b1, b2):
    D = cfg.D
    w1o_e = np.concatenate([np.asarray(W1_root, np.float32),
                            np.asarray(b1, np.float32).reshape(1, D)], axis=0)
    w2o_e = np.concatenate([np.asarray(W2_root, np.float32),
                            np.asarray(b2, np.float32).reshape(1, D)], axis=0)
    return {
        "w1o": w1o_e.astype(NPBF),
        "w2o": w2o_e.astype(NPBF),
        "w2r": np.asarray(W2_rel, dtype=NPBF),
        "iota": np.broadcast_to(np.arange(128, dtype=np.float32),
                                (128, 128)).astype(NPBF).copy(),
        "id64": np.eye(D, dtype=np.float32).astype(NPBF),
    }


def prepare(cfg, inputs):
    x = np.asarray(inputs["x_embeddings"], dtype=np.float32)
    in_maps, prog, g_base = preprocess(
        cfg, x, inputs["edge_index"], inputs["weights"], inputs["batch"],
        inputs["W1_rel"])
    common = _common_inputs(cfg, inputs["W1_root"], inputs["W2_root"],
                            inputs["W2_rel"], inputs["b1"], inputs["b2"])
    for m in in_maps:
        m.update(common)

    key = (cfg.N, cfg.E, tuple(int(k) for k in prog["K_b"]))
    if key not in _CACHE:
        _CACHE[key] = build_nc(cfg, prog)
    return _CACHE[key], in_maps, g_base


def finish(cfg, inputs, g_base, pool_res):
    batch = np.asarray(inputs["batch"], dtype=np.int64)
    counts = np.bincount(batch, minlength=cfg.G).astype(np.float32)
    pooled = np.zeros((cfg.G + 128, cfg.D), dtype=np.float32)
    for c in range(cfg.CORES):
        pooled[g_base[c]:g_base[c] + 128] += pool_res[c]
    pooled = pooled[:cfg.G] / np.maximum(counts, 1.0)[:, None]
    out = pooled @ np.asarray(inputs["Wl_root"], dtype=np.float32)
    out = out + np.asarray(inputs["bl"], dtype=np.float32)
    return out.astype(np.float32)


def run(cfg, inputs, trace=False):
    nc, in_maps, g_base = prepare(cfg, inputs)
    res = run_bass_kernel_spmd(nc, in_maps, core_ids=list(range(cfg.CORES)),
                               trace=trace)
    out = finish(cfg, inputs, g_base,
                 [res.results[c]["pool"] for c in range(cfg.CORES)])
    return out, res


def kernel(**inputs) -> np.ndarray:
    out, _ = run(REAL_CFG, inputs, trace=False)
    return out


# revision 25
# speedup vs baseline: 2.5430x; 2.5430x over previous
"""Trainium2 Bass kernel for a 2-layer GraphConv GNN + mean-pool + linear.

Reference computation (all fp32):
    h1 = leaky_relu(segsum(w*x[src] -> dst) @ W1_rel + x @ W1_root + b1)
    h2 = leaky_relu(segsum(w*h1[src] -> dst) @ W2_rel + h1 @ W2_root + b2)
    pooled = segment_mean(h2, batch, 512)
    out = pooled @ Wl_root + bl            # [512, 8]

Distribution (8 NeuronCores):
    - Nodes in contiguous shards of 12500/core; edges assigned to the core
      owning their dst node; bf16 on-chip, fp32 PSUM accumulation.
    - W_rel is folded into the gather tables: the host premultiplies
      xt = x @ W1_rel into the layer-1 table; layer 1 computes
      ht = h1 @ W2_rel on-chip and stores THAT as the layer-2 table. The
      aggregation matmuls then accumulate the "rel" term directly into the
      z PSUM tile, and one extra matmul adds the root term W_root^T @ x_fm.
    - Gathers use batched InstDMAGatherAnt (int16 indices): edges are sorted
      by (dst block, src row); chunks of 128 edges are grouped into batches
      of <=8 chunks (1024-idx ucode cap) sharing one 32768-row table window,
      amortizing the ~1us SWDGE fixed overhead ~8x vs indirect_dma_start.
      Batches alternate between 2 SWDGE queues: a single in-order queue
      serializes the random-row transfers (~10us/batch measured); two queues
      pipeline them back to the desc-gen rate (~1.4us/batch measured).
    - Scatter-add to dst is a one-hot matmul per chunk (DVE builds
      onehot[e,s] = (iota==dib)*w, TensorE contracts into z PSUM); the bias
      rides in the root matmul via a ones-row in the feature-major activation
      tiles, and leaky-relu is one Act copy + one DVE scalar_tensor_tensor.
    - The h1 AllGather is split in two shard-half pieces (piece-permuted
      row layout): piece 0 overlaps layer-1's second half, and piece 1
      overlaps layer-2's early gather batches, whose 32768-row windows lie
      entirely inside piece 0 and so depend only on the first collective.
    - Per-graph pooling is a one-hot matmul accumulated across blocks; the
      trivial overlap-add + mean + final 64x8 linear run on host.
"""

import math

import numpy as np
import ml_dtypes

import concourse.bacc as bacc
import concourse.mybir as mybir
import concourse.tile as tile
from concourse.bass_utils import run_bass_kernel_spmd

F32 = mybir.dt.float32
B16 = mybir.dt.bfloat16
I16 = mybir.dt.int16
ALU = mybir.AluOpType
ACTF = mybir.ActivationFunctionType
NPBF = ml_dtypes.bfloat16

WL = 32768          # gather window rows (int16 index reach)
GROUP = 8           # blocks per gather group
MAXCH = 8           # max chunks per gather batch (1024-idx ucode cap)


class Cfg:
    def __init__(self, n_nodes, n_edges, d, n_graphs, n_cores=8, block=128):
        self.N = n_nodes
        self.E = n_edges
        self.D = d
        self.G = n_graphs
        self.CORES = n_cores
        self.BLOCK = block
        self.NPC = n_nodes // n_cores                    # real nodes per core
        self.NB = math.ceil(self.NPC / block)            # blocks per core
        self.NPAD = self.NB * block                      # padded nodes/core
        self.PIECE = self.NPAD // 2                      # AllGather piece rows (Q=2)
        self.TROWS = self.NPAD * n_cores                 # table rows
        self.LEAKY = 0.01


REAL_CFG = Cfg(100000, 1250000, 64, 512)


# ---------------------------------------------------------------------------
# Host-side preprocessing
# ---------------------------------------------------------------------------

def _perm_row(cfg, src):
    """Real node id -> piece-permuted table row (vectorized)."""
    c = src // cfg.NPC
    r = src - c * cfg.NPC
    q = r // cfg.PIECE
    rr = r - q * cfg.PIECE
    return q * (cfg.CORES * cfg.PIECE) + c * cfg.PIECE + rr


def preprocess(cfg, x, edge_index, weights, batch, W1_rel):
    N, E, D, CORES = cfg.N, cfg.E, cfg.D, cfg.CORES
    NPC, NB, NPAD, BLOCK = cfg.NPC, cfg.NB, cfg.NPAD, cfg.BLOCK

    src = np.asarray(edge_index[0], dtype=np.int64)
    dst = np.asarray(edge_index[1], dtype=np.int64)
    w = np.asarray(weights, dtype=np.float32)
    batch = np.asarray(batch, dtype=np.int64)

    # --- per-core edge lists sorted by (dst block, permuted src row) -------
    rowp_all = _perm_row(cfg, src)
    c_dst = dst // NPC
    per_core = []
    cnt = np.zeros((CORES, NB), dtype=np.int64)
    for c in range(CORES):
        m = c_dst == c
        s_r, d_r, w_r = rowp_all[m], dst[m] - c * NPC, w[m]
        blk = d_r // BLOCK
        order = np.lexsort((s_r, blk))
        s_r, d_r, w_r, blk = s_r[order], d_r[order], w_r[order], blk[order]
        dib = d_r - blk * BLOCK
        cnt[c] = np.bincount(blk, minlength=NB)
        per_core.append((s_r, dib, w_r, blk))

    # --- shared chunk/batch structure (SPMD: identical program) ------------
    K_b = np.maximum(1, -(-cnt.max(axis=0) // BLOCK))    # chunks per block
    Kmax = int(K_b.max())

    # per-(block, chunk) src-row extents across all cores
    BIG = np.iinfo(np.int64).max
    rmin = np.full((NB, Kmax), BIG, dtype=np.int64)
    rmax = np.full((NB, Kmax), -1, dtype=np.int64)
    for c in range(CORES):
        s_r, dib, w_r, blk = per_core[c]
        first = np.concatenate([[0], np.cumsum(cnt[c])])[:-1]
        rank = np.arange(len(s_r)) - first[blk]
        ck = rank // BLOCK
        np.minimum.at(rmin, (blk, ck), s_r)
        np.maximum.at(rmax, (blk, ck), s_r)

    SPAN_LIM = WL - 512     # safety margin under the int16 reach
    n_groups = math.ceil(NB / GROUP)
    group_blocks = [list(range(g * GROUP, min((g + 1) * GROUP, NB)))
                    for g in range(n_groups)]
    # serpentine k order: odd groups descend k, so the last band of group g
    # borders the same src band as the first batch of group g+1 and batches
    # can merge across group boundaries without blowing the window span
    cand = []
    for g in range(n_groups):
        blocks = group_blocks[g]
        Kg = int(max(K_b[b] for b in blocks))
        ks = range(Kg) if g % 2 == 0 else range(Kg - 1, -1, -1)
        cand.extend((g, b, k) for k in ks for b in blocks if k < K_b[b])

    chunk_col = np.full((NB, Kmax + 1), -1, dtype=np.int64)
    chunk_batch = np.full_like(chunk_col, -1)
    chunk_jpos = np.full_like(chunk_col, -1)
    n_cols = 0
    batches = []
    windows = []
    icols = 0
    state = {"cur": [], "mn": BIG, "mx": -1}

    def flush():
        nonlocal n_cols, icols
        cur = state["cur"]
        if not cur:
            return
        bidx = len(batches)
        for j, (g, b, k) in enumerate(cur):
            chunk_col[b, k] = n_cols + j
            chunk_batch[b, k] = bidx
            chunk_jpos[b, k] = j
        O = 0 if state["mx"] < 0 else min(state["mn"], cfg.TROWS - WL)
        assert state["mx"] - O <= WL - 1
        bt = {"id": bidx, "members": [(b, k) for (g, b, k) in cur],
              "col0": n_cols, "icol0": icols, "nch": len(cur),
              "group": cur[0][0]}
        batches.append(bt)
        windows.append(int(O))
        n_cols += len(cur)
        icols += len(cur) * (BLOCK // 16)
        state["cur"], state["mn"], state["mx"] = [], BIG, -1

    for (g, b, k) in cand:
        mn = rmin[b, k] if rmin[b, k] != BIG else None
        mx = rmax[b, k]
        nmn = state["mn"] if mn is None else min(state["mn"], mn)
        nmx = max(state["mx"], mx)
        if state["cur"] and (len(state["cur"]) >= MAXCH
                            or (nmx >= 0 and nmn != BIG
                                and nmx - nmn > SPAN_LIM)):
            flush()
            nmn = mn if mn is not None else BIG
            nmx = mx
        state["cur"].append((g, b, k))
        state["mn"], state["mx"] = nmn, nmx
    flush()
    groups = [{"blocks": group_blocks[g],
               "batches": [bt for bt in batches if bt["group"] == g]}
              for g in range(n_groups)]
    windows = np.asarray(windows, dtype=np.int64)

    # --- per-core idx/dib/wgt arrays ----------------------------------------
    in_maps = []
    icol_tot = icols
    g_base = batch[np.arange(CORES) * NPC]
    xt = (x.astype(np.float32) @ np.asarray(W1_rel, dtype=np.float32))
    x2 = np.zeros((cfg.TROWS, 2 * D), dtype=NPBF)
    rows_real = _perm_row(cfg, np.arange(N))
    x2[rows_real, :D] = xt.astype(NPBF)

    for c in range(CORES):
        s_r, dib, w_r, blk = per_core[c]
        first = np.concatenate([[0], np.cumsum(cnt[c])])[:-1]
        rank = np.arange(len(s_r)) - first[blk]
        ck = rank // BLOCK
        p = rank % BLOCK
        cols = chunk_col[blk, ck]
        bidx = chunk_batch[blk, ck]
        jpos = chunk_jpos[blk, ck]

        dib_a = np.full((BLOCK, n_cols), 200.0, dtype=np.float32)
        wgt_a = np.zeros((BLOCK, n_cols), dtype=np.float32)
        dib_a[p, cols] = dib.astype(np.float32)
        wgt_a[p, cols] = w_r

        idx16 = np.zeros((16, icol_tot), dtype=np.int16)
        i_lin = jpos * BLOCK + p
        vals = s_r - windows[bidx]
        assert vals.min() >= 0 and vals.max() <= WL - 1
        icolx = np.array([batches[int(bx)]["icol0"] for bx in bidx]) \
            + i_lin // 16
        idx16[i_lin % 16, icolx] = vals.astype(np.int16)
        src16 = np.tile(idx16, (8, 1))

        xs = x[c * NPC:(c + 1) * NPC]
        xT = np.zeros((D + 1, NPAD), dtype=NPBF)
        xT[:D, :NPC] = xs.T.astype(NPBF)
        xT[D, :] = NPBF(1.0)
        gs = np.full(NPAD, -1.0, dtype=np.float32)
        gs[:NPC] = (batch[c * NPC:(c + 1) * NPC] - g_base[c]).astype(np.float32)
        assert gs.max() < 128.0, "graph span per core exceeds 128"
        in_maps.append({
            "x2": x2,
            "xT": xT,
            "src16": src16,
            "dib": dib_a,
            "wgt": wgt_a,
            "gslot": np.ascontiguousarray(
                gs.reshape(NB, BLOCK).T),
        })

    prog = {"groups": groups, "batches": batches, "windows": windows.tolist(),
            "chunk_col": chunk_col, "K_b": K_b, "n_cols": n_cols,
            "icol_tot": icol_tot}
    return in_maps, prog, g_base


# ---------------------------------------------------------------------------
# Bass program
# ---------------------------------------------------------------------------

def build_nc(cfg, prog):
    D, CORES, NB, NPAD = cfg.D, cfg.CORES, cfg.NB, cfg.NPAD
    n_cols, icol_tot = prog["n_cols"], prog["icol_tot"]
    groups, batches, windows = prog["groups"], prog["batches"], prog["windows"]
    chunk_col = prog["chunk_col"]

    nc = bacc.Bacc("TRN2", target_bir_lowering=False, debug=False,
                   num_devices=CORES, dynamic_dma_scratch_size=49152,
                   num_swdge_queues=2)

    x2_d = nc.dram_tensor("x2", [cfg.TROWS, 2 * D], B16, kind="ExternalInput")
    xT_d = nc.dram_tensor("xT", [D + 1, NPAD], B16, kind="ExternalInput")
    src16_d = nc.dram_tensor("src16", [128, icol_tot], I16,
                             kind="ExternalInput")
    dib_d = nc.dram_tensor("dib", [128, n_cols], F32, kind="ExternalInput")
    wgt_d = nc.dram_tensor("wgt", [128, n_cols], F32, kind="ExternalInput")
    gslot_d = nc.dram_tensor("gslot", [128, NB], F32, kind="ExternalInput")
    w1o_d = nc.dram_tensor("w1o", [D + 1, D], B16, kind="ExternalInput")
    w2o_d = nc.dram_tensor("w2o", [D + 1, D], B16, kind="ExternalInput")
    w2r_d = nc.dram_tensor("w2r", [D, D], B16, kind="ExternalInput")
    iota_d = nc.dram_tensor("iota", [128, 128], B16, kind="ExternalInput")
    id64_d = nc.dram_tensor("id64", [D, D], B16, kind="ExternalInput")

    pool_d = nc.dram_tensor("pool", [128, D], F32, kind="ExternalOutput")

    h1_local = nc.dram_tensor("h1_local", [NPAD, 2 * D], B16)
    h1_full = nc.dram_tensor("h1_full", [cfg.TROWS, 2 * D], B16,
                             addr_space="Shared")

    with tile.TileContext(nc) as tc:
        with (
            tc.tile_pool(name="persist", bufs=1) as pp,
            tc.tile_pool(name="work", bufs=6) as wp,
            tc.tile_pool(name="gat", bufs=30) as gp,
            tc.tile_pool(name="psz", bufs=3, space="PSUM") as psz,
            tc.tile_pool(name="psht", bufs=2, space="PSUM") as psht,
            tc.tile_pool(name="pstp", bufs=2, space="PSUM") as pstp,
            tc.tile_pool(name="pool1", bufs=1, space="PSUM") as pool1,
        ):
            xT_s = pp.tile([D + 1, NPAD], B16, tag="xT")
            h1T_s = pp.tile([D + 1, NPAD], B16, tag="h1T")
            src16_s = pp.tile([128, icol_tot], I16, tag="src16")
            dib_s = pp.tile([128, n_cols], F32, tag="dib")
            wgt_s = pp.tile([128, n_cols], F32, tag="wgt")
            gslot_s = pp.tile([128, NB], F32, tag="gslot")
            w1o_s = pp.tile([D + 1, D], B16, tag="w1o")
            w2o_s = pp.tile([D + 1, D], B16, tag="w2o")
            w2r_s = pp.tile([D, D], B16, tag="w2r")
            iota_s = pp.tile([128, 128], B16, tag="iota")
            id64_s = pp.tile([D, D], B16, tag="id64")

            for t, d_ in [(xT_s, xT_d), (gslot_s, gslot_d), (w1o_s, w1o_d),
                          (w2o_s, w2o_d), (w2r_s, w2r_d),
                          (iota_s, iota_d), (id64_s, id64_d)]:
                nc.sync.dma_start(out=t[:], in_=d_[:, :])
            # piecewise loads so the first gathers/onehots start immediately
            for t, d_, nco in [(src16_s, src16_d, icol_tot),
                               (dib_s, dib_d, n_cols),
                               (wgt_s, wgt_d, n_cols)]:
                step = max(256, -(-nco // 8))
                c0 = 0
                first = True
                while c0 < nco:
                    c1 = min(c0 + (128 if first else step), nco)
                    nc.sync.dma_start(out=t[:, c0:c1], in_=d_[:, c0:c1])
                    c0 = c1
                    first = False
            # ones row for the layer-2 root bias trick
            nc.sync.dma_start(out=h1T_s[D:D + 1, :], in_=xT_d[D:D + 1, :])

            pool_ps = pool1.tile([128, D], F32, tag="pool")

            def gather_batches(group, table_d, gtag, col_map):
                for bt in group["batches"]:
                    gb = gp.tile([128, MAXCH, 2 * D], B16, tag=gtag)
                    nch = bt["nch"]
                    O = int(windows[bt["id"]])
                    nc.gpsimd.dma_gather(
                        gb[:, 0:nch, :],
                        table_d[O:O + WL, :],
                        src16_s[:, bt["icol0"]:bt["icol0"] + nch * 8],
                        nch * 128,
                        nch * 128,
                        2 * D,
                        queue_num=bt["id"] % 2,
                    )
                    for j in range(nch):
                        col_map[bt["col0"] + j] = (gb, j)

            def block_agg(z_ps, b, Kb, col_map):
                """accumulate aggregation matmuls for block b into z_ps."""
                for k in range(Kb):
                    col = int(chunk_col[b, k])
                    gb_t, j = col_map[col]
                    oh = wp.tile([128, 128], B16, tag="oh")
                    nc.vector.tensor_scalar(
                        out=oh[:], in0=iota_s[:],
                        scalar1=dib_s[:, col:col + 1],
                        scalar2=wgt_s[:, col:col + 1],
                        op0=ALU.is_equal, op1=ALU.mult)
                    nc.tensor.matmul(
                        out=z_ps[:],
                        lhsT=gb_t[:, j, 0:D],
                        rhs=oh[:],
                        start=(k == 0), stop=False)

            def leaky_to(dst_ap, z_ps):
                # leaky_relu(z) = max(0.01*z, z), bias already folded into z
                zb = wp.tile([D, 128], B16, tag="zb")
                nc.scalar.activation(out=zb[:], in_=z_ps[:], func=ACTF.Copy)
                nc.vector.scalar_tensor_tensor(
                    out=dst_ap, in0=zb[:], scalar=cfg.LEAKY, in1=zb[:],
                    op0=ALU.mult, op1=ALU.max)

            # ---------------- layer 1 ----------------
            tiles = {}
            for g_i, group in enumerate(groups):
                gather_batches(group, x2_d, "gb", tiles)
                for b in group["blocks"]:
                    z_ps = psz.tile([D, 128], F32, tag="z")
                    block_agg(z_ps, b, int(prog["K_b"][b]), tiles)
                    nc.tensor.matmul(out=z_ps[:], lhsT=w1o_s[:],
                                     rhs=xT_s[:, b * 128:(b + 1) * 128],
                                     start=False, stop=True)
                    hsl = h1T_s[0:D, b * 128:(b + 1) * 128]
                    leaky_to(hsl, z_ps)
                    ht_ps = psht.tile([D, 128], F32, tag="ht")
                    nc.tensor.matmul(out=ht_ps[:], lhsT=w2r_s[:], rhs=hsl,
                                     start=True, stop=True)
                    hts = wp.tile([D, 128], B16, tag="hts")
                    nc.scalar.activation(out=hts[:], in_=ht_ps[:],
                                         func=ACTF.Copy)
                    t_ps = pstp.tile([128, D], B16, tag="tp")
                    nc.tensor.transpose(out=t_ps[:], in_=hts[:],
                                        identity=id64_s[:])
                    hnm = wp.tile([128, D], B16, tag="hnm")
                    nc.scalar.activation(out=hnm[:], in_=t_ps[:],
                                         func=ACTF.Copy)
                    nc.sync.dma_start(
                        out=h1_local[b * 128:(b + 1) * 128, 0:D], in_=hnm[:])
                    if (b + 1) * 128 in (cfg.PIECE, NPAD):
                        q = 0 if (b + 1) * 128 == cfg.PIECE else 1
                        nc.gpsimd.collective_compute(
                            "AllGather",
                            ALU.bypass,
                            replica_groups=[list(range(CORES))],
                            ins=[h1_local[q * cfg.PIECE:(q + 1) * cfg.PIECE, :]],
                            outs=[h1_full[q * cfg.PIECE * CORES:
                                          (q + 1) * cfg.PIECE * CORES, :]],
                        )

            # ---------------- layer 2 ----------------
            tiles = {}
            for g_i, group in enumerate(groups):
                gather_batches(group, h1_full, "gb", tiles)
                for b in group["blocks"]:
                    z_ps = psz.tile([D, 128], F32, tag="z")
                    block_agg(z_ps, b, int(prog["K_b"][b]), tiles)
                    nc.tensor.matmul(out=z_ps[:], lhsT=w2o_s[:],
                                     rhs=h1T_s[:, b * 128:(b + 1) * 128],
                                     start=False, stop=True)
                    h2f = wp.tile([D, 128], B16, tag="h2f")
                    leaky_to(h2f[:], z_ps)
                    t_ps = pstp.tile([128, D], B16, tag="tp")
                    nc.tensor.transpose(out=t_ps[:], in_=h2f[:],
                                        identity=id64_s[:])
                    h2nm = wp.tile([128, D], B16, tag="h2nm")
                    nc.scalar.activation(out=h2nm[:], in_=t_ps[:],
                                         func=ACTF.Copy)
                    ph = wp.tile([128, 128], B16, tag="ph")
                    nc.vector.tensor_scalar(
                        out=ph[:], in0=iota_s[:],
                        scalar1=gslot_s[:, b:b + 1], scalar2=None,
                        op0=ALU.is_equal)
                    nc.tensor.matmul(out=pool_ps[:], lhsT=ph[:], rhs=h2nm[:],
                                     start=(b == 0), stop=(b == NB - 1))

            pool_s = wp.tile([128, D], F32, tag="pools")
            nc.scalar.activation(out=pool_s[:], in_=pool_ps[:], func=ACTF.Copy)
            nc.sync.dma_start(out=pool_d[:, :], in_=pool_s[:])

    nc.compile()
    return nc


# ---------------------------------------------------------------------------
# Entry point
# ---------------------------------------------------------------------------

_CACHE = {}


def _common_inputs(cfg, W1_root, W2_root, W2_rel, b1, b2):
    D = cfg.D
    w1o_e = np.concatenate([np.asarray(W1_root, np.float32),
                            np.asarray(b1, np.float32).reshape(1, D)], axis=0)
    w2o_e = np.concatenate([np.asarray(W2_root, np.float32),
                            np.asarray(b2, np.float32).reshape(1, D)], axis=0)
    return {
        "w1o": w1o_e.astype(NPBF),
        "w2o": w2o_e.astype(NPBF),
        "w2r": np.asarray(W2_rel, dtype=NPBF),
        "iota": np.broadcast_to(np.arange(128, dtype=np.float32),
                                (128, 128)).astype(NPBF).copy(),
        "id64": np.eye(D, dtype=np.float32).astype(NPBF),
    }


def prepare(cfg, inputs):
    x = np.asarray(inputs["x_embeddings"], dtype=np.float32)
    in_maps, prog, g_base = preprocess(
        cfg, x, inputs["edge_index"], inputs["weights"], inputs["batch"],
        inputs["W1_rel"])
    common = _common_inputs(cfg, inputs["W1_root"], inputs["W2_root"],
                            inputs["W2_rel"], inputs["b1"], inputs["b2"])
    for m in in_maps:
        m.update(common)

    key = (cfg.N, cfg.E, tuple(int(k) for k in prog["K_b"]))
    if key not in _CACHE:
        _CACHE[key] = build_nc(cfg, prog)
    return _CACHE[key], in_maps, g_base


def finish(cfg, inputs, g_base, pool_res):
    batch = np.asarray(inputs["batch"], dtype=np.int64)
    counts = np.bincount(batch, minlength=cfg.G).astype(np.float32)
    pooled = np.zeros((cfg.G + 128, cfg.D), dtype=np.float32)
    for c in range(cfg.CORES):
        pooled[g_base[c]:g_base[c] + 128] += pool_res[c]
    pooled = pooled[:cfg.G] / np.maximum(counts, 1.0)[:, None]
    out = pooled @ np.asarray(inputs["Wl_root"], dtype=np.float32)
    out = out + np.asarray(inputs["bl"], dtype=np.float32)
    return out.astype(np.float32)


def run(cfg, inputs, trace=False):
    nc, in_maps, g_base = prepare(cfg, inputs)
    res = run_bass_kernel_spmd(nc, in_maps, core_ids=list(range(cfg.CORES)),
                               trace=trace)
    out = finish(cfg, inputs, g_base,
                 [res.results[c]["pool"] for c in range(cfg.CORES)])
    return out, res


def kernel(**inputs) -> np.ndarray:
    out, _ = run(REAL_CFG, inputs, trace=False)
    return out
